# revision 1
# baseline (speedup 1.0000x reference)
"""MLA-v2 (multi-head latent attention) forward pass on 8 Trainium2 NeuronCores.

Sharding: core c -> (batch b = c // 4, head-group g = c % 4, 4 heads each).
Data parallel over batch; tensor parallel over heads (W_Q / W_up_K / W_up_V
column-sharded, W_O row-sharded).  The compressed latent c_kv is computed
replicated per core.  Each core emits a partial (S, D) output; the host sums
the 4 partials per batch (the unshard step for row-parallel W_O).

On-chip layout is fully "transposed" (feature dim on partitions, sequence on
the free axis) so that no activation transposes are ever needed:
  Q^T = W_Q^T @ X^T      (PSUM -> SBUF, RoPE applied in-place)
  c^T = W_dkv^T @ X^T
  K^T = W_upK^T @ c^T    (RoPE in-place)
  V   = (c^T slice)^T @ W_upV          -> natural (s, d) tiles
  S^T[k, q] = (K^T tile)^T @ Q^T       -> softmax over k via matmul tricks
  O'^T = V_aug^T @ exp(S^T)            -> row 64 of V_aug is ones => denominator
  out  = (O^T norm)^T @ W_O            (normalization folded in post-PV)

RoPE trick: columns of W_Q / W_up_K are permuted per head so even/odd pairs
become [32 evens | 32 odds] blocks; dot products are invariant because Q and K
use the same permutation, and V / W_O are untouched.  Rotation is then three
elementwise ops plus a 32-partition-block swap (done by SBUF->SBUF DMA).
Softmax skips the max-subtraction: scores here are bounded (|s| < ~4), far
inside fp32 exp range.
"""

import numpy as np
import ml_dtypes

import concourse.bass as bass
import concourse.bacc as bacc
import concourse.mybir as mybir
import concourse.tile as tile
from concourse.bass_utils import run_bass_kernel_spmd

F32 = mybir.dt.float32
F32R = mybir.dt.float32r
BF16 = mybir.dt.bfloat16

B = 2
S = 2048
D = 1024
H = 16
DH = 64
DC = 256
HPC = 4          # heads per core
GD = HPC * DH    # per-core sharded model dim (256)
N_CORES = 8
NKT = D // 128   # k-tiles over D (8)
NCT = DC // 128  # k-tiles over DC (2)
NST = S // 128   # seq tiles (16)
NQC = S // 512   # 512-wide q chunks (4)


def _build_nc():
    nc = bacc.Bacc("TRN2", target_bir_lowering=False, debug=False,
                   num_devices=N_CORES)

    xt_d = nc.dram_tensor("xt", [D, S], F32R, kind="ExternalInput").ap()
    wq_d = nc.dram_tensor("wq", [D, GD], F32R, kind="ExternalInput").ap()
    wdkv_d = nc.dram_tensor("wdkv", [D, DC], F32R, kind="ExternalInput").ap()
    wupk_d = nc.dram_tensor("wupk", [DC, GD], F32R, kind="ExternalInput").ap()
    wupv_d = nc.dram_tensor("wupv", [DC, GD], F32R, kind="ExternalInput").ap()
    wo_d = nc.dram_tensor("wo", [GD, D], BF16, kind="ExternalInput").ap()
    ctab_d = nc.dram_tensor("ctab", [128, S], F32R, kind="ExternalInput").ap()
    stab_d = nc.dram_tensor("stab", [128, S], F32R, kind="ExternalInput").ap()
    mask_d = nc.dram_tensor("mask", [128, 128], BF16, kind="ExternalInput").ap()
    e0_d = nc.dram_tensor("e0", [1, 128], F32R, kind="ExternalInput").ap()
    e1_d = nc.dram_tensor("e1", [1, 128], F32R, kind="ExternalInput").ap()
    out_d = [nc.dram_tensor(f"out{p}", [S, D], F32, kind="ExternalOutput").ap()
             for p in range(2)]

    with tile.TileContext(nc) as tc:
        with tc.tile_pool(name="sb", bufs=1) as sb, \
             tc.tile_pool(name="psS", bufs=2, space="PSUM") as psS, \
             tc.tile_pool(name="psA", bufs=2, space="PSUM") as psA, \
             tc.tile_pool(name="psB", bufs=1, space="PSUM") as psB:

            # ---- persistent SBUF tensors ----
            wo = sb.tile([128, NCT * D], BF16, tag="wo")
            ctab = sb.tile([128, S], F32R, tag="ctab")
            stab = sb.tile([128, S], F32R, tag="stab")
            mask_t = sb.tile([128, 128], BF16, tag="mask")
            e0_t = sb.tile([1, 128], F32R, tag="e0")
            e1_t = sb.tile([1, 128], F32R, tag="e1")
            qpair = [sb.tile([128, S], F32R, tag=f"q{p}", name=f"qpair{p}") for p in range(2)]
            kpair = [sb.tile([128, S], F32R, tag=f"k{p}", name=f"kpair{p}") for p in range(2)]
            # V_aug: per seq-tile, per head, a contiguous 65-col block:
            # [64 V dims][1.0]  -> lhsT slice = one block (M=65)
            vaug = sb.tile([128, NST * 260], BF16, tag="vaug")
            opair = sb.tile([128, 1024], BF16, tag="opair")
            recip = sb.tile([1, S], F32R, tag="recip")

            nc.sync.dma_start(wo[:].rearrange("p (t n) -> p t n", t=NCT),
                              wo_d.rearrange("(t p) n -> p t n", p=128))
            nc.sync.dma_start(ctab[:], ctab_d)
            nc.sync.dma_start(stab[:], stab_d)
            nc.sync.dma_start(mask_t[:], mask_d)
            nc.sync.dma_start(e0_t[:], e0_d)
            nc.sync.dma_start(e1_t[:], e1_d)
            nc.any.memset(
                vaug[:].rearrange("p (t c) -> p t c", c=65)[:, :, 64:65], 1.0)

            sbp_cm = tc.tile_pool(name="pt", bufs=3)
            sbp = sbp_cm.__enter__()
            sbo_cm = tc.tile_pool(name="ostagep", bufs=1)
            sbo = sbo_cm.__enter__()
            sbf_cm = tc.tile_pool(name="front", bufs=1)
            sbf = sbf_cm.__enter__()
            xt = sbf.tile([128, NKT * S], F32R, tag="xt")
            wq0 = sbf.tile([128, NKT * 128], F32R, tag="wq0")
            sbf1_cm = tc.tile_pool(name="front1", bufs=1)
            sbf1 = sbf1_cm.__enter__()
            wdkv = sbf1.tile([128, NKT * DC], F32R, tag="wdkv")
            wupk = sbf1.tile([128, NCT * GD], F32R, tag="wupk")
            wupv = sbf1.tile([128, NCT * GD], F32R, tag="wupv")
            cpair = [sbf1.tile([128, S], F32R, tag=f"c{p}", name=f"cpair{p}")
                     for p in range(2)]

            for g in range(2):
                nc.sync.dma_start(
                    wq0[:, g * 4 * 128:(g + 1) * 4 * 128].rearrange(
                        "p (t m) -> p t m", t=4),
                    wq_d[g * 512:(g + 1) * 512, 0:128].rearrange(
                        "(t p) m -> p t m", p=128))
                nc.sync.dma_start(
                    wdkv[:, g * 4 * DC:(g + 1) * 4 * DC].rearrange(
                        "p (t m) -> p t m", t=4),
                    wdkv_d[g * 512:(g + 1) * 512, :].rearrange(
                        "(t p) m -> p t m", p=128))
            for g in range(4):
                nc.sync.dma_start(
                    xt[:, g * 2 * S:(g + 1) * 2 * S].rearrange(
                        "p (t s) -> p t s", t=2),
                    xt_d[g * 256:(g + 1) * 256, :].rearrange(
                        "(t p) s -> p t s", p=128))
            nc.sync.dma_start(
                wupk[:].rearrange("p (t m) -> p t m", t=NCT),
                wupk_d.rearrange("(t p) m -> p t m", p=128))
            nc.sync.dma_start(
                wupv[:].rearrange("p (t m) -> p t m", t=NCT),
                wupv_d.rearrange("(t p) m -> p t m", p=128))

            def front_gemm(w_all, w_stride, w_off, dst, pool=None, tag="fa"):
                pool = pool or psA
                for qc in range(NQC):
                    acc = pool.tile([128, 512], F32, tag=tag,
                                    name=f"facc_{w_off}_{qc}")
                    for kt in range(NKT):
                        nc.tensor.matmul(
                            acc[:],
                            w_all[:, kt * w_stride + w_off:
                                  kt * w_stride + w_off + 128],
                            xt[:, kt * S + qc * 512: kt * S + (qc + 1) * 512],
                            start=(kt == 0), stop=(kt == NKT - 1),
                        )
                    nc.vector.tensor_copy(dst[:, qc * 512:(qc + 1) * 512],
                                          acc[:])

            def rope(t, idx):
                y = sbf.tile([128, S], F32R, tag="ry", name=f"ry{idx}",
                             bufs=1)
                for blk in range(4):
                    nc.sync.dma_start(y[blk * 32:(blk + 1) * 32, :],
                                      t[(blk ^ 1) * 32:(blk ^ 1) * 32 + 32, :])
                nc.gpsimd.tensor_tensor(y[:], y[:], stab[:],
                                        mybir.AluOpType.mult)
                nc.vector.tensor_mul(t[:], t[:], ctab[:])
                nc.vector.tensor_add(t[:], t[:], y[:])

            def k_gemm(half):
                for qc in range(NQC):
                    acc = psA.tile([128, 512], F32, tag="fa",
                                   name=f"kacc{half}_{qc}")
                    for kt in range(NCT):
                        nc.tensor.matmul(
                            acc[:],
                            wupk[:, kt * GD + 128 * half:
                                 kt * GD + 128 * (half + 1)],
                            cpair[kt][:, qc * 512:(qc + 1) * 512],
                            start=(kt == 0), stop=(kt == NCT - 1),
                        )
                    nc.vector.tensor_copy(
                        kpair[half][:, qc * 512:(qc + 1) * 512], acc[:])

            # pair-0 front
            front_gemm(wq0, 128, 0, qpair[0])
            rope(qpair[0], 0)
            front_gemm(wdkv, DC, 0, cpair[0])
            front_gemm(wdkv, DC, 128, cpair[1])
            k_gemm(0)
            rope(kpair[0], 2)
            k_gemm(1)

            # ---- V natural tiles -> vaug (bf16, with ones column) ----
            for st in range(NST):
                acc = psA.tile([128, 512], F32, tag="fa", name=f"vacc{st}")
                for kt in range(NCT):
                    nc.tensor.matmul(
                        acc[:, 0:GD],
                        cpair[kt][:, st * 128:(st + 1) * 128],
                        wupv[:, kt * GD:(kt + 1) * GD],
                        start=(kt == 0), stop=(kt == NCT - 1),
                    )
                nc.scalar.copy(
                    vaug[:, st * 260:(st + 1) * 260].rearrange(
                        "p (h c) -> p h c", c=65)[:, :, 0:64],
                    acc[:, 0:GD].rearrange("p (h c) -> p h c", c=64))

            sbf1_cm.__exit__(None, None, None)

            def attn_pair(pair, attn_t, mid=None):
                qh = qpair[pair]
                kh = kpair[pair]
                for c in range(2):
                    if c == 1 and mid is not None:
                        mid()
                    c0, c1 = 1024 * c, 1024 * (c + 1)
                    for sub in range(2):
                        h = 2 * pair + sub
                        po = 64 * sub
                        ot = psB.tile([128, 1024], F32, tag="ot",
                                      name=f"ot{h}_{c}")
                        for i in range(NST):
                            qlo = 128 * i
                            if c1 <= qlo:
                                continue
                            lo = max(qlo, c0)
                            st_t = psS.tile([128, 1024], F32, tag="st",
                                            name=f"stt{h}_{i}_{c}")
                            for bb in range(2):
                                b0, b1 = c0 + 512 * bb, c0 + 512 * (bb + 1)
                                blo = max(lo, b0)
                                if blo >= b1:
                                    continue
                                nc.tensor.matmul(
                                    st_t[:, blo - c0:b1 - c0],
                                    kh[po:po + 64, i * 128:(i + 1) * 128],
                                    qh[po:po + 64, blo:b1],
                                    start=True, stop=True,
                                )
                            pt = sbp.tile([128, 1024], BF16, tag="pt",
                                          name=f"pt{h}_{i}_{c}")
                            nc.scalar.activation(
                                pt[:, lo - c0:1024], st_t[:, lo - c0:1024],
                                mybir.ActivationFunctionType.Exp,
                                scale=float(1.0 / np.sqrt(DH)),
                            )
                            if c0 <= qlo < c1:
                                nc.vector.tensor_mul(
                                    pt[:, qlo - c0:qlo - c0 + 128],
                                    pt[:, qlo - c0:qlo - c0 + 128],
                                    mask_t[:])
                            for bb in range(2):
                                b0, b1 = c0 + 512 * bb, c0 + 512 * (bb + 1)
                                blo = max(lo, b0)
                                if blo >= b1:
                                    continue
                                bank = b0 // 512
                                nc.tensor.matmul(
                                    ot[0:65, blo - c0:b1 - c0],
                                    vaug[:, i * 260 + 65 * h:
                                         i * 260 + 65 * (h + 1)],
                                    pt[:, blo - c0:b1 - c0],
                                    start=(i == 0), stop=(i == 4 * bank + 3),
                                    skip_group_check=True,
                                )
                        with nc.allow_low_precision(reason="recip to f32r"):
                            nc.vector.reciprocal(
                                recip[0:1, sub * 1024:(sub + 1) * 1024],
                                ot[64:65, :])
                        nc.any.tensor_copy(opair[po:po + 64, :], ot[0:64, :])
                    for qc in range(2):
                        rt = psA.tile([128, 512], F32, tag="fa",
                                      name=f"rtt{pair}_{c}_{qc}")
                        nc.tensor.matmul(rt[:], e0_t[:],
                                         recip[0:1, qc * 512:(qc + 1) * 512],
                                         start=True, stop=False)
                        nc.tensor.matmul(
                            rt[:], e1_t[:],
                            recip[0:1, 1024 + qc * 512:1024 + (qc + 1) * 512],
                            start=False, stop=True)
                        nc.any.tensor_mul(
                            attn_t[:, c0 + qc * 512:c0 + (qc + 1) * 512],
                            opair[:, qc * 512:(qc + 1) * 512], rt[:])

            def wo_gemm(p, attn_t, qg_lo=0, qg_hi=NST // 2):
                for qg in range(qg_lo, qg_hi):
                    ostage = sbo.tile([128, 2 * D], F32, tag="ostage",
                                      bufs=2, name=f"ostage{p}_{qg}")
                    for qi in range(2):
                        qt = qg * 2 + qi
                        for nch in range(2):
                            acc = psA.tile([128, 512], F32, tag="fa",
                                           name=f"woacc{p}_{qt}_{nch}")
                            nc.tensor.matmul(
                                acc[:],
                                attn_t[:, qt * 128:(qt + 1) * 128],
                                wo[:, p * D + nch * 512: p * D + (nch + 1) * 512],
                                start=True, stop=True,
                            )
                            nc.any.tensor_copy(
                                ostage[:, qi * D + nch * 512:
                                       qi * D + (nch + 1) * 512],
                                acc[:])
                    eng = nc.sync if qg % 2 == 0 else nc.scalar
                    eng.dma_start(
                        out_d[p][qg * 256:(qg + 1) * 256, :].rearrange(
                            "(t p) n -> p t n", p=128),
                        ostage[:].rearrange("p (t n) -> p t n", t=2))

            # pair-0 attention; pair-1 front fills its gaps
            sbw_cm = tc.tile_pool(name="wq1p", bufs=1)
            sbw = sbw_cm.__enter__()
            wq1 = sbw.tile([128, NKT * 128], F32R, tag="wq1")
            for g in range(2):
                nc.sync.dma_start(
                    wq1[:, g * 4 * 128:(g + 1) * 4 * 128].rearrange(
                        "p (t m) -> p t m", t=4),
                    wq_d[g * 512:(g + 1) * 512, 128:256].rearrange(
                        "(t p) m -> p t m", p=128))
            attn0 = sbp.tile([128, S], BF16, tag="attn", bufs=1, name="attn0")

            def mid0():
                front_gemm(wq1, 128, 0, qpair[1])
                rope(qpair[1], 1)
                rope(kpair[1], 3)
                wo_gemm(0, attn0, 0, 4)

            attn_pair(0, attn0, mid=mid0)
            sbw_cm.__exit__(None, None, None)
            wo_gemm(0, attn0, 4, 8)
            attn1 = sbp.tile([128, S], BF16, tag="attn", bufs=1, name="attn1")
            attn_pair(1, attn1, mid=lambda: wo_gemm(1, attn1, 0, 4))
            wo_gemm(1, attn1, 4, 8)

            sbf_cm.__exit__(None, None, None)
            sbo_cm.__exit__(None, None, None)
            sbp_cm.__exit__(None, None, None)

    nc.compile()
    return nc


_NC_CACHE = []


def _get_nc():
    if not _NC_CACHE:
        _NC_CACHE.append(_build_nc())
    return _NC_CACHE[0]


def _host_tables():
    theta = 10000.0 ** (-np.arange(0, DH, 2, dtype=np.float64) / DH)  # (32,)
    pos = np.arange(S, dtype=np.float64)
    ang = np.outer(theta, pos)  # (32, S)
    cos = np.cos(ang).astype(np.float32)
    sin = np.sin(ang).astype(np.float32)
    ctab = np.tile(cos, (4, 1))  # (128, S)
    stab = np.concatenate([-sin, sin, -sin, sin], axis=0).astype(np.float32)
    mask = (np.arange(512 * 0 + 128)[None, :] >= np.arange(128)[:, None])
    mask = mask.astype(ml_dtypes.bfloat16)  # (128,128): 1 where q >= k
    ee = np.zeros((2, 128), np.float32)
    ee[0, 0:64] = 1.0
    ee[1, 64:128] = 1.0
    return ctab, stab, mask, ee


def _make_in_maps(X, W_Q, W_down_kv, W_up_K, W_up_V, W_O):
    X = np.asarray(X, np.float32)
    W_Q = np.asarray(W_Q, np.float32)
    W_down_kv = np.asarray(W_down_kv, np.float32)
    W_up_K = np.asarray(W_up_K, np.float32)
    W_up_V = np.asarray(W_up_V, np.float32)
    W_O = np.asarray(W_O, np.float32)

    ctab, stab, mask, ee = _host_tables()
    e0, e1 = ee[0:1], ee[1:2]
    # per-head even/odd de-interleave permutation (applied to W_Q and W_up_K
    # output columns; Q.K dot products are invariant under the shared perm)
    perm_head = np.concatenate([np.arange(0, DH, 2), np.arange(1, DH, 2)])

    in_maps = []
    for c in range(N_CORES):
        b, g = c // 4, c % 4
        cols = np.concatenate(
            [(g * HPC + h) * DH + perm_head for h in range(HPC)])
        vcols = slice(g * GD, (g + 1) * GD)
        in_maps.append({
            "xt": np.ascontiguousarray(X[b].T),
            "wq": np.ascontiguousarray(W_Q[:, cols]),
            "wdkv": W_down_kv,
            "wupk": np.ascontiguousarray(W_up_K[:, cols]),
            "wupv": np.ascontiguousarray(W_up_V[:, vcols]),
            "wo": np.ascontiguousarray(W_O[vcols, :]).astype(ml_dtypes.bfloat16),
            "ctab": ctab, "stab": stab, "mask": mask, "e0": e0, "e1": e1,
        })

    return in_maps


def _gather(res):
    out = np.zeros((B, S, D), np.float32)
    for c in range(N_CORES):
        out[c // 4] += res.results[c]["out0"]
        out[c // 4] += res.results[c]["out1"]
    return out


def kernel(X, W_Q, W_down_kv, W_up_K, W_up_V, W_O):
    in_maps = _make_in_maps(X, W_Q, W_down_kv, W_up_K, W_up_V, W_O)
    nc = _get_nc()
    res = run_bass_kernel_spmd(nc, in_maps, core_ids=list(range(N_CORES)))
    return _gather(res)



# revision 11
# speedup vs baseline: 1.5363x; 1.5363x over previous
"""MLA-v2 (multi-head latent attention) forward pass on 8 Trainium2 NeuronCores.

Sharding: core c -> (batch b = c // 4, head-group g = c % 4, 4 heads each).
Data parallel over batch; tensor parallel over heads (W_Q / W_up_K / W_up_V
column-sharded, W_O row-sharded).  The compressed latent c_kv is computed
replicated per core.  Each core emits one bf16 (S, D) partial; the host sums
the 4 partials per batch (the unshard step for row-parallel W_O).

Layout highlights (v2):
  * All activations/weights in bf16 (fp32 PSUM accumulation); Q/K stored as
    fp8e4m3 so QK^T runs in DoubleRow mode (0.5 PE cycles/row).
  * Q^T/K^T layout [128 part = 4 heads x 32 pair-idx, 2*S free]: even dh
    components in cols [0,S), odd in [S,2S).  RoPE becomes pure free-dim
    elementwise ops (no partition swaps); QK^T contracts (32 part x 2
    subtiles) per head via DoubleRow.
  * PV in (q, d) orientation: lhsT = probs tile, rhs = V-natural augmented
    with a ones column => full-rate PE + per-partition softmax denominators.
  * attn (q, d) -> (d, q) via DMA XBAR transpose (no PE/vector cost).
  * Front gemms (Q, c_kv, K, V) pipelined per 512-col slab with attention
    chunks (256 q) and the W_O gemm, to overlap PE work with the
    activation-engine exp stream.
"""

import numpy as np
import ml_dtypes

import concourse.bass as bass
import concourse.bacc as bacc
import concourse.mybir as mybir
import concourse.tile as tile
from concourse.bass_utils import run_bass_kernel_spmd

F32 = mybir.dt.float32
BF16 = mybir.dt.bfloat16
FP8 = mybir.dt.float8e4
DR = mybir.MatmulPerfMode.DoubleRow
EXP = mybir.ActivationFunctionType.Exp

B = 2
S = 2048
D = 1024
H = 16
DH = 64
DC = 256
HPC = 4          # heads per core
GD = HPC * DH    # per-core sharded model dim (256)
N_CORES = 8
NKT = D // 128   # k-tiles over D (8)
NCT = DC // 128  # k-tiles over DC (2)
NST = S // 128   # seq tiles (16)
NQC = S // 512   # 512-wide front slabs (4)
NCH = S // 256   # 256-wide attention chunks (8)
SCALE = 1.0 / float(np.sqrt(DH))


def _build_nc():
    nc = bacc.Bacc("TRN2", target_bir_lowering=False, debug=False,
                   num_devices=N_CORES)

    xt_d = nc.dram_tensor("xt", [D, S], BF16, kind="ExternalInput").ap()
    wq_d = nc.dram_tensor("wq", [D, 256], BF16, kind="ExternalInput").ap()
    wdkv_d = nc.dram_tensor("wdkv", [D, DC], BF16, kind="ExternalInput").ap()
    wupk_d = nc.dram_tensor("wupk", [DC, 256], BF16, kind="ExternalInput").ap()
    wupv_d = nc.dram_tensor("wupv", [DC, GD], BF16, kind="ExternalInput").ap()
    wo_d = nc.dram_tensor("wo", [GD, D], BF16, kind="ExternalInput").ap()
    ctab_d = nc.dram_tensor("ctab", [128, 2 * S], BF16, kind="ExternalInput").ap()
    stab_d = nc.dram_tensor("stab", [128, 2 * S], BF16, kind="ExternalInput").ap()
    mask_d = nc.dram_tensor("maskt", [128, 128], BF16, kind="ExternalInput").ap()
    out_d = nc.dram_tensor("out0", [S, D], BF16, kind="ExternalOutput").ap()

    with tile.TileContext(nc) as tc:
        with tc.tile_pool(name="sb", bufs=1) as sb, \
             tc.tile_pool(name="sbt", bufs=1) as sbt, \
             tc.tile_pool(name="psS", bufs=2, space="PSUM") as psS, \
             tc.tile_pool(name="psP", bufs=2, space="PSUM") as psP, \
             tc.tile_pool(name="psA", bufs=2, space="PSUM") as psA:

            # ---- persistent SBUF tensors ----
            xt = sb.tile([128, NKT * S], BF16, tag="xt")
            wq = sb.tile([128, NKT * 256], BF16, tag="wq")
            wdkv = sb.tile([128, NKT * DC], BF16, tag="wdkv")
            wupk = sb.tile([128, NCT * 256], BF16, tag="wupk")
            wupv = sb.tile([128, NCT * GD], BF16, tag="wupv")
            wo = sb.tile([128, NCT * D], BF16, tag="wo")
            ctab = sb.tile([128, 2 * S], BF16, tag="ctab")
            stab = sb.tile([128, 2 * S], BF16, tag="stab")
            maskt = sb.tile([128, 128], BF16, tag="maskt")
            qtmp = sb.tile([128, 2 * S], BF16, tag="qtmp")
            ktmp = sb.tile([128, 2 * S], BF16, tag="ktmp")
            qf8 = sb.tile([128, 2 * S], FP8, tag="qf8")
            kf8 = sb.tile([128, 2 * S], FP8, tag="kf8")
            # DoubleRow matmuls crash when consecutive instructions use
            # different PE row tile positions, so every head's Q/K operand is
            # staged at partition base 0: [32 pair-idx, head * (2, S)]
            qf8w = sb.tile([32, HPC * 2 * S], FP8, tag="qf8w")
            kf8w = sb.tile([32, HPC * 2 * S], FP8, tag="kf8w")
            cpair = sb.tile([128, NCT * S], BF16, tag="cpair")
            vaug = sb.tile([128, NST * 260], BF16, tag="vaug")
            attn_T = sb.tile([128, NCT * S], BF16, tag="attn_T")
            recips = sb.tile([128, NST * HPC], F32, tag="recips")

            qf8_r = qf8[:].rearrange("p (t s) -> p t s", t=2)
            kf8_r = kf8[:].rearrange("p (t s) -> p t s", t=2)
            qf8w_r = qf8w[:].rearrange("p (h t s) -> p h t s", h=HPC, t=2)
            kf8w_r = kf8w[:].rearrange("p (h t s) -> p h t s", h=HPC, t=2)

            def qk_ops(h):
                return kf8w_r[:, h], qf8w_r[:, h]

            def stage_qk(w_r, f_r, qc):
                for h in range(HPC):
                    nc.sync.dma_start(
                        w_r[:, h, :, qc * 512:(qc + 1) * 512],
                        f_r[32 * h:32 * h + 32, :, qc * 512:(qc + 1) * 512])

            # ---- input DMAs (order = DMA_ENGINES service order) ----
            nc.sync.dma_start(
                wq[:].rearrange("p (t n) -> p t n", t=NKT),
                wq_d.rearrange("(t p) n -> p t n", p=128))
            nc.sync.dma_start(
                wdkv[:].rearrange("p (t n) -> p t n", t=NKT),
                wdkv_d.rearrange("(t p) n -> p t n", p=128))

            def dma_x(qc):
                nc.sync.dma_start(
                    xt[:].rearrange("p (t s) -> p t s", t=NKT)[
                        :, :, qc * 512:(qc + 1) * 512],
                    xt_d.rearrange("(t p) s -> p t s", p=128)[
                        :, :, qc * 512:(qc + 1) * 512])

            dma_x(0)
            nc.sync.dma_start(ctab[:], ctab_d)
            nc.sync.dma_start(stab[:], stab_d)
            nc.sync.dma_start(
                wupk[:].rearrange("p (t n) -> p t n", t=NCT),
                wupk_d.rearrange("(t p) n -> p t n", p=128))
            nc.sync.dma_start(
                wupv[:].rearrange("p (t n) -> p t n", t=NCT),
                wupv_d.rearrange("(t p) n -> p t n", p=128))
            dma_x(1)
            nc.sync.dma_start(maskt[:], mask_d)
            nc.sync.dma_start(
                wo[:].rearrange("p (t n) -> p t n", t=NCT),
                wo_d.rearrange("(t p) n -> p t n", p=128))
            dma_x(2)
            dma_x(3)

            # ones columns of V-augmented tiles (denominator accumulators)
            nc.any.memset(
                vaug[:].rearrange("p (q u) -> p q u", u=65)[:, :, 64:65], 1.0)

            def front_chain(dst, dst_off, w, w_stride, w_off, nkt,
                            rhs, rhs_stride, qc, name):
                acc = psA.tile([128, 512], F32, tag="fa", name=name)
                for kt in range(nkt):
                    nc.tensor.matmul(
                        acc[:],
                        w[:, kt * w_stride + w_off:kt * w_stride + w_off + 128],
                        rhs[:, kt * rhs_stride + qc * 512:
                            kt * rhs_stride + (qc + 1) * 512],
                        start=(kt == 0), stop=(kt == nkt - 1),
                    )
                nc.any.tensor_copy(dst[:, dst_off:dst_off + 512], acc[:])

            def rope_piece(src, dst, qc, nm):
                # src bf16 [128, 2S] (evens | odds); dst fp8 same layout
                src3 = src[:].rearrange("p (t s) -> p t s", t=2)[
                    :, :, qc * 512:(qc + 1) * 512]
                dst3 = dst[:].rearrange("p (t s) -> p t s", t=2)[
                    :, :, qc * 512:(qc + 1) * 512]
                c3 = ctab[:].rearrange("p (t s) -> p t s", t=2)[
                    :, :, qc * 512:(qc + 1) * 512]
                s3 = stab[:].rearrange("p (t s) -> p t s", t=2)[
                    :, :, qc * 512:(qc + 1) * 512]
                y = sbt.tile([128, 1024], BF16, tag="ry", bufs=2,
                             name=f"ry{nm}")
                y3 = y[:].rearrange("p (t s) -> p t s", t=2, s=512)
                nc.any.tensor_copy(y3[:, 0, :], src3[:, 1, :])
                nc.any.tensor_copy(y3[:, 1, :], src3[:, 0, :])
                nc.any.tensor_mul(src3, src3, c3)
                nc.any.tensor_mul(y3, y3, s3)
                nc.any.tensor_add(dst3, src3, y3)

            def wo_gemm(qg):
                ost = sbt.tile([128, 1024], BF16, tag="ost", bufs=2,
                               name=f"ost{qg}")
                for nch in range(2):
                    accw = psA.tile([128, 512], F32, tag="fa",
                                    name=f"wo{qg}_{nch}")
                    for dblk in range(2):
                        nc.tensor.matmul(
                            accw[:],
                            attn_T[:, dblk * S + qg * 128:
                                   dblk * S + (qg + 1) * 128],
                            wo[:, dblk * D + nch * 512:
                               dblk * D + (nch + 1) * 512],
                            start=(dblk == 0), stop=(dblk == 1),
                        )
                    nc.any.tensor_copy(ost[:, nch * 512:(nch + 1) * 512],
                                       accw[:])
                nc.sync.dma_start(out_d[qg * 128:(qg + 1) * 128, :], ost[:])

            pending_wo = []

            def attn_chunk(c):
                npair = c + 1
                pts = []
                for p_i in range(npair):
                    for hp in range(2):
                        st_t = psS.tile([128, 1024], F32, tag="st",
                                        name=f"st{c}_{p_i}_{hp}")
                        pt = sbt.tile([128, 1024], BF16, tag="pt", bufs=16,
                                      name=f"pt{c}_{p_i}_{hp}")
                        for t in range(2):
                            kt = 2 * p_i + t
                            lo = max(0, 128 * (kt - 2 * c))
                            for hl in range(2):
                                h = 2 * hp + hl
                                k_op, q_op = qk_ops(h)
                                nc.tensor.matmul(
                                    st_t[:, t * 512 + hl * 256 + lo:
                                         t * 512 + (hl + 1) * 256],
                                    k_op[:, :, kt * 128:(kt + 1) * 128],
                                    q_op[:, :, c * 256 + lo:(c + 1) * 256],
                                    start=(hl == 0), stop=(hl == 1),
                                    perf_mode=DR, skip_group_check=True,
                                )
                        if p_i < c:
                            nc.scalar.activation(pt[:], st_t[:], EXP,
                                                 scale=SCALE)
                        else:
                            # diagonal pair: exp only the causal region
                            nc.scalar.activation(pt[:, 0:512],
                                                 st_t[:, 0:512], EXP,
                                                 scale=SCALE)
                            in3 = st_t[:, 512:1024].rearrange(
                                "p (hl q) -> p hl q", q=256)[:, :, 128:256]
                            out3 = pt[:, 512:1024].rearrange(
                                "p (hl q) -> p hl q", q=256)[:, :, 128:256]
                            nc.scalar.activation(out3, in3, EXP, scale=SCALE)
                            for hl in range(2):
                                off = hl * 256
                                nc.any.tensor_mul(pt[:, off:off + 128],
                                                  pt[:, off:off + 128],
                                                  maskt[:])
                                off = 512 + hl * 256 + 128
                                nc.any.tensor_mul(pt[:, off:off + 128],
                                                  pt[:, off:off + 128],
                                                  maskt[:])
                        pts.append(pt)

                for qsl in range(2):
                    qg = 2 * c + qsl
                    acc = psP.tile([128, 512], F32, tag="pv",
                                   name=f"pv{qg}")
                    for p_i in range(npair):
                        for t in range(2):
                            kt = 2 * p_i + t
                            if kt > qg:
                                continue
                            for hp in range(2):
                                pt = pts[p_i * 2 + hp]
                                for hl in range(2):
                                    h = 2 * hp + hl
                                    nc.tensor.matmul(
                                        acc[:, h * 65:(h + 1) * 65],
                                        pt[:, t * 512 + hl * 256 + qsl * 128:
                                           t * 512 + hl * 256 + (qsl + 1) * 128],
                                        vaug[:, kt * 260 + h * 65:
                                             kt * 260 + (h + 1) * 65],
                                        start=(kt == 0 and h == 0),
                                        stop=(kt == qg and h == 3),
                                        skip_group_check=True,
                                    )
                    # softmax denominators -> reciprocals
                    nc.vector.reciprocal(
                        recips[:, qg * 4:qg * 4 + 4].rearrange(
                            "p (h u) -> p h u", u=1),
                        acc[:, 64:64 + 4 * 65].rearrange(
                            "p (h u) -> p h u", u=65)[:, :, 0:1])
                    anat = sbt.tile([128, 256], BF16, tag="anat", bufs=2,
                                    name=f"anat{qg}")
                    for h in range(HPC):
                        nc.any.tensor_scalar(
                            anat[:, h * 64:(h + 1) * 64],
                            acc[:, h * 65:h * 65 + 64],
                            recips[:, qg * 4 + h:qg * 4 + h + 1], None,
                            mybir.AluOpType.mult)
                    nc.sync.dma_start_transpose(
                        attn_T[:].rearrange("p (d s) -> p d s", d=2)[
                            :, :, qg * 128:(qg + 1) * 128],
                        anat[:])
                    pending_wo.append(qg)

            # ---- pipelined front + attention + WO ----
            for qc in range(NQC):
                for eo in range(2):
                    front_chain(qtmp, eo * S + qc * 512, wq, 256, eo * 128,
                                NKT, xt, S, qc, f"q{eo}_{qc}")
                for ct in range(2):
                    front_chain(cpair, ct * S + qc * 512, wdkv, DC, ct * 128,
                                NKT, xt, S, qc, f"c{ct}_{qc}")
                rope_piece(qtmp, qf8, qc, f"q{qc}")
                stage_qk(qf8w_r, qf8_r, qc)
                for eo in range(2):
                    front_chain(ktmp, eo * S + qc * 512, wupk, 256, eo * 128,
                                NCT, cpair, S, qc, f"k{eo}_{qc}")
                rope_piece(ktmp, kf8, qc, f"k{qc}")
                stage_qk(kf8w_r, kf8_r, qc)
                for st in range(4 * qc, 4 * qc + 4):
                    accv = psA.tile([128, 512], F32, tag="fa",
                                    name=f"v{st}")
                    for ct in range(2):
                        nc.tensor.matmul(
                            accv[:, 0:GD],
                            cpair[:, ct * S + st * 128:ct * S + (st + 1) * 128],
                            wupv[:, ct * GD:(ct + 1) * GD],
                            start=(ct == 0), stop=(ct == 1),
                        )
                    nc.any.tensor_copy(
                        vaug[:, st * 260:(st + 1) * 260].rearrange(
                            "p (h u) -> p h u", u=65)[:, :, 0:64],
                        accv[:, 0:GD].rearrange("p (h u) -> p h u", u=64))

                for c in (2 * qc, 2 * qc + 1):
                    # trail the W_O gemm by one chunk so the XBAR transpose
                    # is never on the PE critical path
                    while len(pending_wo) > 2:
                        wo_gemm(pending_wo.pop(0))
                    attn_chunk(c)
            while pending_wo:
                wo_gemm(pending_wo.pop(0))

    nc.compile()
    return nc


_NC_CACHE = []


def _get_nc():
    if not _NC_CACHE:
        _NC_CACHE.append(_build_nc())
    return _NC_CACHE[0]


def _host_tables():
    theta = 10000.0 ** (-np.arange(0, DH, 2, dtype=np.float64) / DH)  # (32,)
    pos = np.arange(S, dtype=np.float64)
    ang = np.outer(theta, pos)  # (32, S)
    cos = np.cos(ang)
    sin = np.sin(ang)
    cos4 = np.tile(cos, (4, 1))  # (128, S) rows = (head, pair-idx)
    sin4 = np.tile(sin, (4, 1))
    ctab = np.concatenate([cos4, cos4], axis=1).astype(ml_dtypes.bfloat16)
    stab = np.concatenate([-sin4, sin4], axis=1).astype(ml_dtypes.bfloat16)
    mask = (np.arange(128)[None, :] >= np.arange(128)[:, None])
    mask = mask.astype(ml_dtypes.bfloat16)  # (128,128): 1 where q >= k
    return ctab, stab, mask


def _make_in_maps(X, W_Q, W_down_kv, W_up_K, W_up_V, W_O):
    X = np.asarray(X, np.float32)
    W_Q = np.asarray(W_Q, np.float32)
    W_down_kv = np.asarray(W_down_kv, np.float32)
    W_up_K = np.asarray(W_up_K, np.float32)
    W_up_V = np.asarray(W_up_V, np.float32)
    W_O = np.asarray(W_O, np.float32)

    ctab, stab, mask = _host_tables()
    bf = ml_dtypes.bfloat16

    in_maps = []
    for c in range(N_CORES):
        b, g = c // 4, c % 4
        cols_e = np.concatenate(
            [(g * HPC + h) * DH + np.arange(0, DH, 2) for h in range(HPC)])
        cols_o = cols_e + 1
        cols = np.concatenate([cols_e, cols_o])  # (256,): [4h evens | 4h odds]
        vcols = slice(g * GD, (g + 1) * GD)
        in_maps.append({
            "xt": np.ascontiguousarray(X[b].T).astype(bf),
            "wq": np.ascontiguousarray(W_Q[:, cols]).astype(bf),
            "wdkv": W_down_kv.astype(bf),
            "wupk": np.ascontiguousarray(W_up_K[:, cols]).astype(bf),
            "wupv": np.ascontiguousarray(W_up_V[:, vcols]).astype(bf),
            "wo": np.ascontiguousarray(W_O[vcols, :]).astype(bf),
            "ctab": ctab, "stab": stab, "maskt": mask,
        })

    return in_maps


def _gather(res):
    out = np.zeros((B, S, D), np.float32)
    for c in range(N_CORES):
        out[c // 4] += res.results[c]["out0"].astype(np.float32)
    return out


def kernel(X, W_Q, W_down_kv, W_up_K, W_up_V, W_O):
    in_maps = _make_in_maps(X, W_Q, W_down_kv, W_up_K, W_up_V, W_O)
    nc = _get_nc()
    res = run_bass_kernel_spmd(nc, in_maps, core_ids=list(range(N_CORES)))
    return _gather(res)
